# revision 4
# baseline (speedup 1.0000x reference)
"""Single-head causal attention with RoPE on 8 TRN2 NeuronCores — v2.3.

Sharding: core c -> batch c//2, parity p = c%2 takes the interleaved
512-row q-blocks {p, p+2, p+4, p+6} of T=4096 (causal load balance).
Each core computes full K/V for its batch.

Design vs original baseline:
- All inputs bf16 (host-cast): halves HBM traffic.
- K/Q projected directly transposed ([d, t] PSUM out, W stationary,
  512-col sweeps); RoPE applied along the partition (d) axis on DVE,
  batched over 512 t, written straight to SBUF bf16.
- V projected naturally ([t, d], x-tile stationary) into [s, 129]
  tiles whose 129th column is a baked-in 1.0: each AV matmul
  (Pt-block stationary, [s,129] moving) emits both the output block
  and the softmax denominator -> no separate row-sum sweep, no
  output transposes.
- Output accumulates as [q, 129] PSUM regions; normalization is an
  ACT copy with per-partition scale = 1/denominator.
- start=True zeroes a whole 2KB PSUM bank, so only bank-leading
  regions (qt 0/2) set it and epilogue reads bank-leaders last.
- Attention rebalanced across the kv-slot stream: q-slot j+1's
  full-block tiles run during step j (right after q(j+1) is roped),
  only the diag+tail tiles and epilogue of q-slot j run in step j.
  Tail after the last projection shrinks from 122 to 8 tiles.
- DMA: 256KB pair-packed x loads, descriptor issue spread over
  SP/ACT/GPSIMD engines (~600ns each), weights loaded per-ctile so
  the first matmul starts ~8us in.
"""
import numpy as np
import ml_dtypes

B, T, C, HD = 4, 4096, 2048, 128
P = 128
NB = 8          # 512-row blocks per sequence
BS = 512        # block size
NC16 = C // P   # 16 c-tiles
SCALE = float(C) ** -0.5
NEG = -1.0e9
BF16 = ml_dtypes.bfloat16
VS = 132        # stride of one [s, 129] v tile (pad to keep 4B align)


def build():
    import concourse.bass as bass
    import concourse.mybir as mybir
    import bass_rust
    from concourse.tile import TileContext

    f32 = mybir.dt.float32
    bf16 = mybir.dt.bfloat16
    EXP = mybir.ActivationFunctionType.Exp
    COPY = mybir.ActivationFunctionType.Copy

    nc = bass.Bass()
    # host layouts (slot-ordered t):
    # xt: [8 slots][8 pair-tiles][128, 1024] bf16
    #     pair g row p = [xT[2g*128+p, 512 t] | xT[(2g+1)*128+p, 512 t]]
    xt = nc.declare_dram_parameter("xt", [NB * 8 * P, 2 * BS], bf16,
                                   isOutput=False)
    # w: [16 ctiles][128 c, 384] bf16, cols = [k(128, perm) | q(128, perm) | v(128)]
    w = nc.declare_dram_parameter("w", [NC16 * P, 384], bf16, isOutput=False)
    # csT/snT: [128 d, 4096 t] fp32, slot-ordered t, rotate-half structure
    csT = nc.declare_dram_parameter("csT", [P, T], bf16, isOutput=False)
    snT = nc.declare_dram_parameter("snT", [P, T], bf16, isOutput=False)
    tailb = nc.declare_dram_parameter("tailb", [P, 1], f32, isOutput=False)
    out = nc.declare_dram_parameter("out", [T // 2, HD], bf16, isOutput=True)

    H = 64

    with TileContext(nc) as tc:
        with (
            tc.tile_pool(name="const", bufs=1) as cp,
            tc.tile_pool(name="xp", bufs=4) as xp,
            tc.tile_pool(name="rs", bufs=2) as rsp,      # rope scratch
            tc.tile_pool(name="ptp", bufs=3) as ptp,     # rotating Pt tiles
            tc.tile_pool(name="osb", bufs=4) as osb,     # out staging + rcp
            tc.tile_pool(name="pps", bufs=2, space="PSUM") as pps,   # k/q proj
            tc.tile_pool(name="sps", bufs=2, space="PSUM") as sps,   # scores + v
            tc.tile_pool(name="ops", bufs=1, space="PSUM") as ops,   # o2+sm x2
        ):
            # ---- residents ----
            wt = cp.tile([P, NC16 * 384], bf16, tag="wt")
            cst = cp.tile([P, T], bf16, tag="cst")
            snt = cp.tile([P, T], bf16, tag="snt")
            kT = cp.tile([P, T], bf16, tag="kT")        # roped K^T [d, t]
            qT = cp.tile([P, T // 2], bf16, tag="qT")   # roped Q^T [d, t own]
            vs = cp.tile([P, 32 * VS], bf16, tag="vs")  # V [s,129], col128=1
            tri = cp.tile([P, P], f32, tag="tri")
            tb = cp.tile([P, 1], f32, tag="tb")
            # persistent o2 accumulators: q-slot j uses o2ab[j % 2]
            o2ab = [ops.tile([P, 1024], f32, tag="o2a", name="o2a"),
                    ops.tile([P, 1024], f32, tag="o2b", name="o2b")]

            def load_cssn(sl, eng):
                c = slice(sl * BS, (sl + 1) * BS)
                eng.dma_start(cst[:, c], csT[:, c])
                eng.dma_start(snt[:, c], snT[:, c])

            # weights: one DMA per ctile (96KB); ci=0 lands first so the
            # first projection matmul starts ~8us in
            for ci in range(NC16):
                nc.scalar.dma_start(wt[:, ci * 384:(ci + 1) * 384],
                                    w[ci * P:(ci + 1) * P, :])
            load_cssn(0, nc.gpsimd)
            load_cssn(4, nc.gpsimd)
            nc.sync.dma_start(tb[:], tailb[:])

            # ---- x loads: pair-packed tiles, two 128KB DMAs each so a
            # single queue never serializes more than ~7us of one slot ----
            def load_slot(sl):
                xts = []
                for g in range(8):
                    xtile = xp.tile([P, 2 * BS], bf16, tag=f"x{g}")
                    r0 = (sl * 8 + g) * P
                    for h in range(2):
                        eng = nc.sync if (2 * g + h) % 4 < 2 else nc.gpsimd
                        eng.dma_start(xtile[:, h * BS:(h + 1) * BS],
                                      xt[r0:r0 + P, h * BS:(h + 1) * BS])
                    xts.append(xtile)
                return xts

            def xsl(xts, ci, cols):
                # [c-tile ci] columns `cols` of the pair-packed tiles
                base = (ci % 2) * BS
                return xts[ci // 2][:, base + cols.start: base + cols.stop]

            def init_consts():
                # tri01[s, q] = 1.0 where s <= q else 0.0 (within 128 block)
                nc.gpsimd.memset(tri[:], 0.0)
                nc.gpsimd.affine_select(
                    out=tri[:], in_=tri[:],
                    compare_op=mybir.AluOpType.is_gt,
                    fill=1.0, base=0,
                    pattern=[[-1, P]], channel_multiplier=1,
                )
                # tri_neg = (tri01 - 1) * 1e9: 0 on causal, -1e9 off
                nc.gpsimd.tensor_scalar_add(tri[:], tri[:], -1.0)
                nc.gpsimd.tensor_scalar_mul(tri[:], tri[:], 1.0e9)
                for s128 in range(32):
                    nc.vector.memset(
                        vs[:, s128 * VS + P: s128 * VS + P + 1], 1.0)

            def rope_d(pp, dst, tcols):
                """pp: [d,512] PSUM f32 -> dst bf16 SBUF slice."""
                tmp = rsp.tile([P, BS], f32, tag="rtmp")
                prod = rsp.tile([P, BS], f32, tag="rprod")
                nc.vector.tensor_mul(tmp[0:H, :], pp[H:P, :], snt[0:H, tcols])
                nc.vector.tensor_mul(tmp[H:P, :], pp[0:H, :], snt[H:P, tcols])
                nc.vector.tensor_mul(prod[:], pp[:], cst[:, tcols])
                nc.vector.tensor_add(dst, prod[:], tmp[:])

            def proj_kq(sl, xts, which):
                tcols = slice(sl * BS, (sl + 1) * BS)
                off = 0 if which == "k" else P
                pp = pps.tile([P, BS], f32, tag="pp")
                for ci in range(NC16):
                    nc.tensor.matmul(
                        pp[:], wt[:, ci * 384 + off: ci * 384 + off + P],
                        xsl(xts, ci, slice(0, BS)),
                        start=(ci == 0), stop=(ci == NC16 - 1))
                dst = kT if which == "k" else qT
                rope_d(pp, dst[:, tcols], tcols)

            def proj_v(sl, xts):
                for tt in range(4):
                    vp = sps.tile([P, P], f32, tag="S")
                    for ci in range(NC16):
                        nc.tensor.matmul(
                            vp[:], xsl(xts, ci, slice(tt * P, (tt + 1) * P)),
                            wt[:, ci * 384 + 2 * P: ci * 384 + 3 * P],
                            start=(ci == 0), stop=(ci == NC16 - 1))
                    s128 = sl * 4 + tt
                    nc.vector.tensor_copy(
                        vs[:, s128 * VS: s128 * VS + P], vp[:])

            # ---- attention tiles (software-pipelined by one stage so the
            # 4 AV matmuls of tile i never clog the PE wait queue while
            # exp(i) is still running: S/exp of tile i+1 issues first) ----
            def emit_s_exp(j, si, kind, st):
                s128 = si * 4 + st
                c0 = st * P if kind == "diag" else 0
                Sp = sps.tile([P, BS], f32, tag="S")
                nc.tensor.matmul(
                    Sp[:, c0:BS], kT[:, s128 * P:(s128 + 1) * P],
                    qT[:, j * BS + c0: (j + 1) * BS],
                    start=True, stop=True)
                if kind == "diag":
                    nc.vector.tensor_add(
                        Sp[:, c0:c0 + P], Sp[:, c0:c0 + P], tri[:])
                ptile = ptp.tile([P, BS], bf16, tag="Pt")
                bias = tb[:, 0:1] if kind == "tail" else 0.0
                nc.scalar.activation(ptile[:, c0:BS], Sp[:, c0:BS],
                                     EXP, bias=bias, scale=SCALE)
                return ptile

            def emit_avs(j, si, kind, st, ptile):
                o2 = o2ab[j % 2]
                s128 = si * 4 + st
                qt0 = st if kind == "diag" else 0
                first = ((j == 0 and kind == "diag" and st == 0)
                         or (j > 0 and kind == "full" and si == 0 and st == 0))
                last = (kind == "tail" and st == 3)
                for qt in range(qt0, 4):
                    # start=True zeroes the whole 2KB PSUM bank: only
                    # bank-leading regions (qt 0, 2) set it.
                    nc.tensor.matmul(
                        o2[:, qt * 256: qt * 256 + 129],
                        ptile[:, qt * P:(qt + 1) * P],
                        vs[:, s128 * VS: s128 * VS + 129],
                        start=(first and qt % 2 == 0), stop=last,
                        skip_group_check=True)

            def attn_step(step):
                # all of q-slot `step`'s tiles: fulls, then diag, then tail.
                # Attention load grows with step, intentionally matching DMA
                # supply: x streaming dominates early steps, attention late.
                tiles = []
                for si in list(range(step)) + [4 + s for s in range(step)]:
                    for st in range(4):
                        tiles.append((step, si, "full", st))
                for st in range(4):
                    tiles.append((step, step, "diag", st))
                for st in range(4):
                    tiles.append((step, 4 + step, "tail", st))
                pend = None
                for tl in tiles:
                    pt = emit_s_exp(*tl)
                    if pend:
                        emit_avs(*pend)
                    pend = (*tl, pt)
                emit_avs(*pend)
                epilogue(step)

            def epilogue(j):
                o2 = o2ab[j % 2]
                # rcps, then scaled copies, then stores.
                # bank-leading regions (qt 0, 2) read LAST per engine so
                # their WAR dep orders the bank-mates' reads before the
                # next start's bank-wide zero.
                rcps, ots = {}, {}
                for qt in (1, 0, 3, 2):
                    rcps[qt] = osb.tile([P, 1], f32, tag="rcp",
                                        name=f"rcp{qt}")
                    nc.vector.reciprocal(
                        rcps[qt][:], o2[:, qt * 256 + 128: qt * 256 + 129])
                for qt in (1, 0, 3, 2):
                    ots[qt] = osb.tile([P, P], bf16, tag="ot", name=f"ot{qt}")
                    nc.scalar.activation(
                        ots[qt][:], o2[:, qt * 256: qt * 256 + P],
                        COPY, bias=0.0, scale=rcps[qt][:, 0:1])
                for qt in range(4):
                    r0 = j * BS + qt * P
                    nc.sync.dma_start(out[r0:r0 + P, :], ots[qt][:])

            # ---- main schedule ----
            xq = {0: load_slot(0), 4: load_slot(4), 1: load_slot(1)}
            init_consts()
            load_cssn(1, nc.scalar)
            load_cssn(5, nc.scalar)
            for step in range(4):
                sl = step
                xts_a = xq.pop(sl)
                xts_b = xq.pop(sl + 4)
                # q first: attention's full-block scores only need qT
                proj_kq(sl, xts_a, "q")
                proj_kq(sl, xts_a, "k")
                proj_kq(sl + 4, xts_b, "k")
                proj_v(sl, xts_a)
                proj_v(sl + 4, xts_b)
                if step < 3:
                    xq[step + 5] = load_slot(step + 5)
                    xq[step + 2] = load_slot(step + 2)
                if step < 2:
                    load_cssn(step + 2, nc.scalar)
                    load_cssn(step + 6, nc.scalar)
                attn_step(step)

    bass_rust.generate_event_semaphores(nc)
    return nc


_CACHE = {}


def _get_nc():
    if "nc" not in _CACHE:
        _CACHE["nc"] = build()
    return _CACHE["nc"]


def _prep_inputs(x, Wq, Wk, Wv, cos, sin):
    perm = np.concatenate([np.arange(0, HD, 2), np.arange(1, HD, 2)])
    wkT = Wk[perm].T.astype(np.float32)     # [C, 128]
    wqT = Wq[perm].T.astype(np.float32)
    wvT = Wv.T.astype(np.float32)
    wfull = np.concatenate([wkT, wqT, wvT], axis=1)       # [C, 384]
    wtiles = wfull.reshape(NC16, P, 384).astype(BF16)
    wflat = np.ascontiguousarray(wtiles.reshape(NC16 * P, 384))

    cos2 = np.concatenate([cos, cos], axis=1).astype(np.float32)  # [T, 128]
    sin2 = np.concatenate([-sin, sin], axis=1).astype(np.float32)

    in_maps = []
    orders = []
    for c in range(8):
        b, par = c // 2, c % 2
        order = [par, par + 2, par + 4, par + 6,
                 1 - par, 3 - par, 5 - par, 7 - par]
        orders.append(order)
        xb = np.asarray(x[b], np.float32)          # [T, C]
        xtiles = np.empty((NB, NC16, P, BS), BF16)
        c2 = np.empty((P, T), BF16)
        s2 = np.empty((P, T), BF16)
        for sl, ab in enumerate(order):
            rows = slice(ab * BS, (ab + 1) * BS)
            blk = xb[rows].T.astype(BF16)          # [C, 512]
            xtiles[sl] = blk.reshape(NC16, P, BS)
            c2[:, sl * BS:(sl + 1) * BS] = cos2[rows].T
            s2[:, sl * BS:(sl + 1) * BS] = sin2[rows].T
        # pair-pack: [8 slots][8 pairs][128, 1024]
        xpk = xtiles.reshape(NB, 8, 2, P, BS).transpose(0, 1, 3, 2, 4)
        tailv = np.full((P, 1), NEG if par == 0 else 0.0, np.float32)
        in_maps.append({
            "xt": np.ascontiguousarray(xpk.reshape(NB * 8 * P, 2 * BS)),
            "w": wflat,
            "csT": np.ascontiguousarray(c2),
            "snT": np.ascontiguousarray(s2),
            "tailb": tailv,
        })
    return in_maps, orders


def _run(x, Wq, Wk, Wv, cos, sin, trace=False):
    from concourse.bass_utils import run_bass_kernel_spmd
    nc = _get_nc()
    in_maps, orders = _prep_inputs(x, Wq, Wk, Wv, cos, sin)
    res = run_bass_kernel_spmd(nc, in_maps, list(range(8)), trace=trace)
    full = np.empty((B, T, HD), np.float32)
    for c in range(8):
        b, order = c // 2, orders[c]
        oc = np.asarray(res.results[c]["out"], dtype=np.float32)
        for j in range(4):
            ab = order[j]
            full[b, ab * BS:(ab + 1) * BS] = oc[j * BS:(j + 1) * BS]
    return full, res


def kernel(x, Wq, Wk, Wv, cos, sin):
    return _run(x, Wq, Wk, Wv, cos, sin, trace=False)[0]
